# revision 35
# baseline (speedup 1.0000x reference)
"""Trainium2 Bass kernel for nn_Attention_63513976373985.

Strategy: pure data-parallel over the batch dim B=64 across 8 NeuronCores
(8 batches per core, all params replicated, no collectives). Inside each
core, per-batch pipeline:
  X = d2[b]            [S=512, F=512]  fp16 in DRAM, pre-transposed on the
                       host to [f, s] so loads are plain (cheap) DMAs
  d3T = relu(w1.T @ XT + b1)           [C, S]   (fp16 matmul, f32 PSUM)
  tv  = tanh(XT.T @ wv)                [S, C]   natural layout for vs
  per head h: zsT = Wtop[h].T @ d3T  (+ zconst[h,b] per-partition ACT bias,
              where zconst = relu(d1@w1+b1) @ Wbot[h] — the d4 half of d5)
              usT = tanh(zsT + zconst)           [C, S]
  atts = blockdiag(P) matvec over usT            [H, S]
  softmax over S (ACT exp w/ accum_out, DVE reciprocal; 1/Z folded into
  the vs eviction as a per-partition scale)
  scoresT via PE transpose, vs = scoresT.T @ tv  [H, C]
  V slabs via PE transpose, outT = relu(wcc.T @ V + bcc)  [128, BLOC]

Scheduling (the perf-critical part; 182us -> ~163us):
  - d2 is pre-transposed on the HOST so xt loads are plain DMAs instead
    of xbar DMA-transposes (the transposes cost ~1.26us of queue time
    each and dominated the startup ramp).
  - All DMAs ride the sync queue in near-consumption order. The DMA
    semaphore-slot pool is recycled globally across queues: a new DMA
    stalls until the previous transfer on its slot completed, so
    issuing DMAs out of consumption order (or on the scalar queue,
    ahead of ACT work) measured 20-70us SLOWER.
  - The head loop is software-pipelined: atts(h-1) is issued after
    zs(h), so the PE never waits on the ACT tanh eviction.
  - The softmax-dependent tail of batch b (scoresT transpose, vs, vsT)
    is issued after d3(b+1) on the PE, hiding the cross-engine softmax
    latency under the next batch's matmuls.
  - Final projection is wcc-stationary producing outT [128, BLOC] so bcc
    is a per-partition ACT bias (host un-transposes); output columns
    0..6 project during batch 7 (column b depends only on batch b),
    and batch 7 runs heads->tv so tv+projection fill its softmax gap.
  - PSUM: pmm 2 + pzs 4 + pat 1 + psmall 1 = 8 banks exactly.

Everything runs fp16 (same 10-bit mantissa as tf32; fp32 PSUM
accumulate). fp8 (DoubleRow, 2x PE) was measured in numpy sim to blow
the error budget on the atts path: rel 8.6e-2 vs the 2e-2 gate.
LDWEIGHTS fully hides under matmuls (dual weight buffers), so
weight-stationary restructuring buys nothing; matmul cost is
~165ns issue overhead + N*0.4167ns, N<=512 per PSUM bank.
"""
import sys

if "/opt/trn_rl_repo" not in sys.path:
    sys.path.insert(0, "/opt/trn_rl_repo")

import numpy as np

H, F, C, S, B = 8, 512, 256, 512, 64
NCORES = 8
BLOC = B // NCORES  # 8
OUTF = 128

_CACHE = {}


def build_nc():
    import concourse.bass as bass  # noqa: F401
    import concourse.mybir as mybir
    import concourse.tile as tile
    from concourse import bacc
    from contextlib import ExitStack

    f32 = mybir.dt.float32
    f16 = mybir.dt.float16
    AF = mybir.ActivationFunctionType

    nc = bacc.Bacc("TRN2", target_bir_lowering=False, debug=False,
                   num_devices=NCORES)

    # ---- DRAM parameters (per-core shard shapes) ----
    # d2 arrives host-pre-transposed per batch: [b, f-in-tile, s-half,
    # kf, s-in-half] — s-half-major so each half is a 2KB/partition DMA
    d2_d = nc.dram_tensor("d2", [BLOC, 128, 2, 4, S // 2], f16,
                          kind="ExternalInput")
    d1t_d = nc.dram_tensor("d1t", [128, 4, BLOC], f16, kind="ExternalInput")
    w1_d = nc.dram_tensor("w1r", [128, 2, 4, 128], f16, kind="ExternalInput")
    wv_d = nc.dram_tensor("wvr", [128, 4, C], f16, kind="ExternalInput")
    wtop_d = nc.dram_tensor("wtopr", [128, H, 2, 2, 128], f16, kind="ExternalInput")
    wbot_d = nc.dram_tensor("wbotr", [128, H, 2, 2, 128], f16, kind="ExternalInput")
    pblk_d = nc.dram_tensor("pblkr", [128, 2 * H, H], f16, kind="ExternalInput")
    wcc_d = nc.dram_tensor("wccr", [128, 2 * H, OUTF], f16, kind="ExternalInput")
    bcct_d = nc.dram_tensor("bcct", [128, 1], f32, kind="ExternalInput")
    b1c_d = nc.dram_tensor("b1c", [128, 2], f32, kind="ExternalInput")
    id8_d = nc.dram_tensor("id8", [8, 8], f16, kind="ExternalInput")
    out_d = nc.dram_tensor("out", [OUTF, BLOC], f32, kind="ExternalOutput")

    with tile.TileContext(nc) as tc, ExitStack() as stk:
        const = stk.enter_context(tc.tile_pool(name="const", bufs=1))
        xtp = stk.enter_context(tc.tile_pool(name="xtp", bufs=3))
        d3p = stk.enter_context(tc.tile_pool(name="d3p", bufs=3))
        tvpool = stk.enter_context(tc.tile_pool(name="tvpool", bufs=2))
        usp = stk.enter_context(tc.tile_pool(name="usp", bufs=4))
        smallsb = stk.enter_context(tc.tile_pool(name="smallsb", bufs=2))
        vpool = stk.enter_context(tc.tile_pool(name="vpool", bufs=1))
        pmm = stk.enter_context(tc.tile_pool(name="pmm", bufs=2, space="PSUM"))
        pzs = stk.enter_context(tc.tile_pool(name="pzs", bufs=4, space="PSUM"))
        patp = stk.enter_context(tc.tile_pool(name="patp", bufs=1, space="PSUM"))
        psmall = stk.enter_context(
            tc.tile_pool(name="psmall", bufs=1, space="PSUM"))

        # ---- DMA issue: spread across queues so useful DMAs start the
        # moment the framework preamble ends. HWDGE (transpose-capable)
        # queues are sync+scalar only; gpsimd software-DGE carries the
        # small consts. wbot/wtop are single-issue (contiguous layout) —
        # per-DMA queue-issue cost is ~620ns regardless of size.
        def load_xt(b, name):
            xt = xtp.tile([128, 2, 4, S // 2], f16, tag="xt", name=name)
            for sh in range(2):
                nc.sync.dma_start(
                    out=xt[:, sh, :, :],
                    in_=d2_d[b, :, sh, :, :])
            return xt

        # All DMAs ride the sync queue in near-consumption order (the DMA
        # semaphore-slot recycling couples queues globally; spreading DMAs
        # across queues measured SLOWER — slot waits blocked the ACT
        # stream). Only change vs baseline: w1/d1t lead, so the d4
        # prologue and d3(b0) start as early as possible.
        xt_tiles = {}
        # Startup is DMA-bandwidth-bound: the gpsimd SWDGE path (~107GB/s)
        # transfers concurrently with the sync HWDGE path, so the ~3.5MB
        # of startup bytes are split across both, each in consumption
        # order (slot recycling blocks on same-slot transfer completion).
        w1_sb = const.tile([128, 2, 4, 128], f16, tag="w1")
        xtp0 = xtp.tile([128, 2, 4, S // 2], f16, tag="xt", name="xt_pre0")
        nc.gpsimd.dma_start(out=xtp0[:, 0, :, :], in_=d2_d[0, :, 0, :, :])
        nc.sync.dma_start(out=w1_sb[:, 0, :, :], in_=w1_d[:, 0, :, :])
        nc.gpsimd.dma_start(out=xtp0[:, 1, :, :], in_=d2_d[0, :, 1, :, :])
        nc.sync.dma_start(out=w1_sb[:, 1, :, :], in_=w1_d[:, 1, :, :])
        xt_tiles[0] = xtp0
        d1t_sb = const.tile([128, 4, BLOC], f16, tag="d1t")
        nc.gpsimd.dma_start(out=d1t_sb, in_=d1t_d[:, :, :])
        wv_sb = const.tile([128, 4, C], f16, tag="wv")
        nc.sync.dma_start(out=wv_sb, in_=wv_d[:, :, :])
        b1c_sb = const.tile([128, 2], f32, tag="b1c")
        nc.gpsimd.dma_start(out=b1c_sb, in_=b1c_d[:, :])
        wbot_sb = const.tile([128, H, 2, 2, 128], f16, tag="wbot")
        nc.gpsimd.dma_start(out=wbot_sb, in_=wbot_d[:, :, :, :, :])
        wtop_sb = const.tile([128, H, 2, 2, 128], f16, tag="wtop")
        for h in range(H):
            nc.sync.dma_start(out=wtop_sb[:, h, :, :, :],
                              in_=wtop_d[:, h, :, :, :])
        pblk_sb = const.tile([128, 2 * H, H], f16, tag="pblk")
        nc.gpsimd.dma_start(out=pblk_sb, in_=pblk_d[:, :, :])
        id8_sb = const.tile([8, 8], f16, tag="id8")
        nc.sync.dma_start(out=id8_sb, in_=id8_d[:, :])
        xt_tiles[1] = load_xt(1, "xt_pre1")
        bcct_sb = const.tile([128, 1], f32, tag="bcct")
        nc.gpsimd.dma_start(out=bcct_sb, in_=bcct_d[:, :])
        wcc_sb = const.tile([128, 2 * H, OUTF], f16, tag="wcc")

        # ---- per-batch stage emitters ----
        d3ts = {}
        tvs = {}
        escs = {}
        zinvs = {}

        def emit_d3(b):
            xt = xt_tiles[b]
            d3t = d3p.tile([128, 2, S], f16, tag="d3t", name=f"d3t{b}")
            if b == 0:
                # batch 0 splits by s-half so compute starts after only
                # the first 256KB of xt0 lands
                for sh in range(2):
                    for m in range(2):
                        pmd3 = pmm.tile([128, S // 2], f32, tag="mm",
                                        name=f"pmd3_0_{m}_{sh}")
                        for kf in range(4):
                            nc.tensor.matmul(pmd3, lhsT=w1_sb[:, m, kf, :],
                                             rhs=xt[:, sh, kf, :],
                                             start=(kf == 0), stop=(kf == 3))
                        nc.scalar.activation(
                            d3t[:, m, sh * 256:(sh + 1) * 256], pmd3,
                            AF.Relu, bias=b1c_sb[:, m:m + 1])
            else:
                for m in range(2):
                    pmd3 = pmm.tile([128, S], f32, tag="mm",
                                    name=f"pmd3_{b}_{m}")
                    for kf in range(4):
                        nc.tensor.matmul(pmd3, lhsT=w1_sb[:, m, kf, :],
                                         rhs=xt[:, :, kf, :],
                                         start=(kf == 0), stop=(kf == 3))
                    nc.scalar.activation(d3t[:, m, :], pmd3, AF.Relu,
                                         bias=b1c_sb[:, m:m + 1])
            d3ts[b] = d3t

        def emit_tv(b):
            xt = xt_tiles[b]
            tv = tvpool.tile([128, 4, C], f16, tag="tv", name=f"tv{b}")
            for mp in range(2):
                pmtv = pmm.tile([128, 2, C], f32, tag="mm",
                                name=f"pmtv_{b}_{mp}")
                for ms2 in range(2):
                    ms = mp * 2 + ms2
                    for kf in range(4):
                        nc.tensor.matmul(
                            pmtv[:, ms2, :],
                            lhsT=xt[:, ms // 2, kf,
                                    (ms % 2) * 128:(ms % 2 + 1) * 128],
                            rhs=wv_sb[:, kf, :],
                            start=(kf == 0), stop=(kf == 3))
                nc.scalar.activation(tv[:, mp * 2:(mp + 1) * 2, :], pmtv,
                                     AF.Tanh)
            tvs[b] = tv

        def emit_heads(b):
            """zs/us/atts with atts one head behind zs (software pipeline),
            then the softmax front (nmax/exp/recip) on DVE+ACT."""
            d3t = d3ts[b]
            pat = patp.tile([8, S], f32, tag="atts", name=f"pat{b}")
            us_tiles = {}

            def emit_zs(h):
                us = usp.tile([128, 2, S], f16, tag="us", name=f"us{b}_{h}")
                for ct in range(2):
                    pz = pzs.tile([128, S], f32, tag="zs",
                                  name=f"pz{b}_{h}_{ct}")
                    for ks in range(2):
                        nc.tensor.matmul(pz, lhsT=wtop_sb[:, h, ks, ct, :],
                                         rhs=d3t[:, ks, :],
                                         start=(ks == 0), stop=(ks == 1))
                    nc.scalar.activation(us[:, ct, :], pz, AF.Tanh,
                                         bias=zc_sb[:, ct, h, b:b + 1])
                us_tiles[h] = us

            def emit_atts(h):
                us = us_tiles[h]
                for ct in range(2):
                    nc.tensor.matmul(pat, lhsT=pblk_sb[:, h * 2 + ct, :],
                                     rhs=us[:, ct, :],
                                     start=(h == 0 and ct == 0),
                                     stop=(h == H - 1 and ct == 1))

            emit_zs(0)
            for h in range(1, H):
                emit_zs(h)
                emit_atts(h - 1)
            emit_atts(H - 1)

            nmax = smallsb.tile([8, 1], f32, tag="nmax", name=f"nmax{b}")
            nc.vector.tensor_reduce(nmax, pat, axis=mybir.AxisListType.X,
                                    op=mybir.AluOpType.max, negate=True)
            esc = smallsb.tile([8, S], f16, tag="esc", name=f"esc{b}")
            zsum = smallsb.tile([8, 1], f32, tag="zsum", name=f"zsum{b}")
            nc.scalar.activation(esc, pat, AF.Exp, bias=nmax, accum_out=zsum)
            zinv = smallsb.tile([8, 1], f32, tag="zinv", name=f"zinv{b}")
            nc.vector.reciprocal(zinv, zsum)
            escs[b] = esc
            zinvs[b] = zinv

        def emit_tail(b):
            """scoresT transpose + vs + vsT into the V slab. Issued during
            batch b+1 so the softmax latency hides under d3(b+1)."""
            esc, zinv, tv = escs[b], zinvs[b], tvs[b]
            psc = psmall.tile([128, 4, 8], f16, tag="small", name=f"psc{b}")
            for sc in range(4):
                nc.tensor.transpose(psc[:, sc, :],
                                    in_=esc[:, sc * 128:(sc + 1) * 128],
                                    identity=id8_sb)
            sct = smallsb.tile([128, 4, 8], f16, tag="sct", name=f"sct{b}")
            nc.vector.tensor_copy(out=sct, in_=psc)
            pvs = psmall.tile([8, C], f32, tag="small", name=f"pvs{b}")
            for sc in range(4):
                nc.tensor.matmul(pvs, lhsT=sct[:, sc, :],
                                 rhs=tv[:, sc, :],
                                 start=(sc == 0), stop=(sc == 3))
            vssb = smallsb.tile([8, C], f16, tag="vssb", name=f"vssb{b}")
            nc.vector.tensor_scalar_mul(vssb, pvs, zinv)
            pvt = psmall.tile([128, 2, 8], f16, tag="small", name=f"pvt{b}")
            for ch in range(2):
                nc.tensor.transpose(pvt[:, ch, :],
                                    in_=vssb[:, ch * 128:(ch + 1) * 128],
                                    identity=id8_sb)
            for ch in range(2):
                nc.vector.tensor_copy(out=v_sb[:, ch, :, b:b + 1],
                                      in_=pvt[:, ch, :])

        # ---- V accumulator across the b loop ----
        v_sb = vpool.tile([128, 2, H, BLOC], f16)  # [c-in-half, ch, h, b]

        # ---- batch 0: d3 first (needs only xt0+w1, the earliest DMAs),
        # then the d4/zconst prologue (needs d1t/wbot, landing later).
        emit_d3(0)

        # d4T = relu(w1.T @ d1T + b1) : [C(2 tiles), BLOC]
        pd4 = psmall.tile([128, 2, BLOC], f32, tag="small")
        for m in range(2):
            for k in range(4):
                nc.tensor.matmul(pd4[:, m, :], lhsT=w1_sb[:, m, k, :],
                                 rhs=d1t_sb[:, k, :],
                                 start=(k == 0), stop=(k == 3))
        d4t_sb = const.tile([128, 2, BLOC], f16, tag="d4t")
        for m in range(2):
            nc.scalar.activation(d4t_sb[:, m, :], pd4[:, m, :], AF.Relu,
                                 bias=b1c_sb[:, m:m + 1])

        emit_tv(0)

        pzc = psmall.tile([128, 2, H, BLOC], f32, tag="small")
        for ct in range(2):
            for h in range(H):
                for ks in range(2):
                    nc.tensor.matmul(pzc[:, ct, h, :],
                                     lhsT=wbot_sb[:, h, ks, ct, :],
                                     rhs=d4t_sb[:, ks, :],
                                     start=(ks == 0), stop=(ks == 1))
        zc_sb = const.tile([128, 2, H, BLOC], f32, tag="zc")
        nc.vector.tensor_copy(out=zc_sb, in_=pzc)

        emit_heads(0)

        outsb = smallsb.tile([OUTF, BLOC], f32, tag="outsb")

        def emit_final(bs, be, name):
            """outT[:, bs:be] = relu(wcc.T @ V[:, bs:be] + bccT). Column b
            of the output depends only on batch b, so batches 0..6 project
            during batch 7's compute and only b7's column remains at the
            end."""
            pout = psmall.tile([OUTF, be - bs], f32, tag="small", name=name)
            kidx = 0
            for h in range(H):
                for ch in range(2):
                    nc.tensor.matmul(pout, lhsT=wcc_sb[:, h * 2 + ch, :],
                                     rhs=v_sb[:, ch, h, bs:be],
                                     start=(kidx == 0), stop=(kidx == 15))
                    kidx += 1
            nc.scalar.activation(outsb[:, bs:be], pout, AF.Relu,
                                 bias=bcct_sb[:, 0:1])

        for b in range(1, BLOC):
            if b + 1 < BLOC:
                xt_tiles[b + 1] = load_xt(b + 1, f"xt{b + 1}")
            if b == 2:
                nc.sync.dma_start(out=wcc_sb, in_=wcc_d[:, :, :])
            emit_d3(b)
            emit_tail(b - 1)
            if b < BLOC - 1:
                emit_tv(b)
                emit_heads(b)
        # last batch: heads first, then tv + the b0..6 projection fill the
        # softmax latency; only b7's rank-1 column remains after tail(7).
        emit_heads(BLOC - 1)
        emit_tv(BLOC - 1)
        emit_final(0, BLOC - 1, "pout_a")
        emit_tail(BLOC - 1)
        emit_final(BLOC - 1, BLOC, "pout_b")
        nc.sync.dma_start(out=out_d[:, :], in_=outsb)

    nc.compile()
    return nc


def host_inputs(d1, d2, w1, b1, W, P, wv, wcc, bcc):
    """Host-side sharding + layout prep. Returns in_maps for 8 cores."""
    d1 = np.ascontiguousarray(d1, dtype=np.float32)
    d2 = np.ascontiguousarray(d2, dtype=np.float32)
    w1 = np.ascontiguousarray(w1, dtype=np.float32)
    b1 = np.ascontiguousarray(b1, dtype=np.float32)
    W = np.ascontiguousarray(W, dtype=np.float32)
    P = np.ascontiguousarray(P, dtype=np.float32)
    wv = np.ascontiguousarray(wv, dtype=np.float32)
    wcc = np.ascontiguousarray(wcc, dtype=np.float32)
    bcc = np.ascontiguousarray(bcc, dtype=np.float32)

    w1r = np.ascontiguousarray(
        w1.reshape(4, 128, 2, 128).transpose(1, 2, 0, 3))
    wvr = np.ascontiguousarray(wv.reshape(4, 128, C).transpose(1, 0, 2))
    wtopr = np.ascontiguousarray(
        W[:, :C, :].reshape(H, 2, 128, 2, 128).transpose(2, 0, 1, 3, 4))
    wbotr = np.ascontiguousarray(
        W[:, C:, :].reshape(H, 2, 128, 2, 128).transpose(2, 0, 1, 3, 4))
    pblkr = np.zeros((128, 2 * H, H), np.float32)
    for h in range(H):
        for ct in range(2):
            pblkr[:, h * 2 + ct, h] = P[h, ct * 128:(ct + 1) * 128]
    wccr = np.ascontiguousarray(
        wcc.reshape(2 * H, 128, OUTF).transpose(1, 0, 2))
    bcct = np.ascontiguousarray(bcc[:, None])
    b1c = np.ascontiguousarray(b1.reshape(2, 128).T)
    id8 = np.eye(8, dtype=np.float32)

    f16 = np.float16
    shared = dict(w1r=w1r.astype(f16), wvr=wvr.astype(f16),
                  wtopr=wtopr.astype(f16), wbotr=wbotr.astype(f16),
                  pblkr=pblkr.astype(f16), wccr=wccr.astype(f16),
                  bcct=bcct, b1c=b1c, id8=id8.astype(f16))
    # pre-transpose d2 on host:
    # [S, B, F] -> per-core [b, f-in-tile, s-half, kf, s-in-half]
    d2t = np.ascontiguousarray(
        d2.transpose(1, 2, 0).astype(np.float16))  # [B, F, S]
    in_maps = []
    for core in range(NCORES):
        bs = slice(core * BLOC, (core + 1) * BLOC)
        d2c = np.ascontiguousarray(
            d2t[bs].reshape(BLOC, 4, 128, 2, 256).transpose(0, 2, 3, 1, 4))
        d1c = d1[bs]  # [BLOC, F]
        d1tr = np.ascontiguousarray(
            d1c.T.reshape(4, 128, BLOC).transpose(1, 0, 2)).astype(np.float16)
        in_maps.append(dict(d2=d2c, d1t=d1tr, **shared))
    return in_maps


def kernel(**inputs):
    if "nc" not in _CACHE:
        _CACHE["nc"] = build_nc()
    nc = _CACHE["nc"]
    in_maps = host_inputs(
        d1=inputs["d1"], d2=inputs["d2"], w1=inputs["w1"], b1=inputs["b1"],
        W=inputs["W"], P=inputs["P"], wv=inputs["wv"], wcc=inputs["wcc"],
        bcc=inputs["bcc"])
    from concourse.bass_utils import run_bass_kernel_spmd
    res = run_bass_kernel_spmd(nc, in_maps, core_ids=list(range(NCORES)))
    return np.concatenate(
        [res.results[i]["out"].T for i in range(NCORES)], axis=0)


# revision 36
# speedup vs baseline: 1.0383x; 1.0383x over previous
"""Trainium2 Bass kernel for nn_Attention_63513976373985.

Strategy: pure data-parallel over the batch dim B=64 across 8 NeuronCores
(8 batches per core, all params replicated, no collectives). Inside each
core, per-batch pipeline:
  X = d2[b]            [S=512, F=512]  fp16 in DRAM, pre-transposed on the
                       host to [f, s] so loads are plain (cheap) DMAs
  d3T = relu(w1.T @ XT + b1)           [C, S]   (fp16 matmul, f32 PSUM)
  tv  = tanh(XT.T @ wv)                [S, C]   natural layout for vs
  per head h: zsT = Wtop[h].T @ d3T  (+ zconst[h,b] per-partition ACT bias,
              where zconst = relu(d1@w1+b1) @ Wbot[h] — the d4 half of d5)
              usT = tanh(zsT + zconst)           [C, S]
  atts = blockdiag(P) matvec over usT            [H, S]
  softmax over S (ACT exp w/ accum_out, DVE reciprocal; 1/Z folded into
  the vs eviction as a per-partition scale)
  scoresT via PE transpose, vs = scoresT.T @ tv  [H, C]
  V slabs via PE transpose, outT = relu(wcc.T @ V + bcc)  [128, BLOC]

Scheduling (the perf-critical part; 182us -> ~163us):
  - d2 is pre-transposed on the HOST so xt loads are plain DMAs instead
    of xbar DMA-transposes (the transposes cost ~1.26us of queue time
    each and dominated the startup ramp).
  - All DMAs ride the sync queue in near-consumption order. The DMA
    semaphore-slot pool is recycled globally across queues: a new DMA
    stalls until the previous transfer on its slot completed, so
    issuing DMAs out of consumption order (or on the scalar queue,
    ahead of ACT work) measured 20-70us SLOWER. Splitting startup DMAs
    onto the gpsimd SWDGE path also measured slower (~+5us).
  - The head loop is software-pipelined: atts(h-1) is issued after
    zs(h), so the PE never waits on the ACT tanh eviction.
  - The softmax-dependent tail of batch b (scoresT transpose, vs, vsT)
    is issued after d3(b+1) on the PE, hiding the cross-engine softmax
    latency under the next batch's matmuls.
  - Final projection is wcc-stationary producing outT [128, BLOC] so bcc
    is a per-partition ACT bias (host un-transposes); output columns
    0..6 project during batch 7 (column b depends only on batch b),
    and batch 7 runs heads->tv so tv+projection fill its softmax gap.
  - PSUM: pmm 2 + pzs 4 + pat 1 + psmall 1 = 8 banks exactly.

Everything runs fp16 (same 10-bit mantissa as tf32; fp32 PSUM
accumulate). fp8 (DoubleRow, 2x PE) was measured in numpy sim to blow
the error budget on the atts path: rel 8.6e-2 vs the 2e-2 gate.
LDWEIGHTS fully hides under matmuls (dual weight buffers), so
weight-stationary restructuring buys nothing; matmul cost is
~165ns issue overhead + N*0.4167ns, N<=512 per PSUM bank.
"""
import sys

if "/opt/trn_rl_repo" not in sys.path:
    sys.path.insert(0, "/opt/trn_rl_repo")

import numpy as np

H, F, C, S, B = 8, 512, 256, 512, 64
NCORES = 8
BLOC = B // NCORES  # 8
OUTF = 128

_CACHE = {}


def build_nc():
    import concourse.bass as bass  # noqa: F401
    import concourse.mybir as mybir
    import concourse.tile as tile
    from concourse import bacc
    from contextlib import ExitStack

    f32 = mybir.dt.float32
    f16 = mybir.dt.float16
    AF = mybir.ActivationFunctionType

    nc = bacc.Bacc("TRN2", target_bir_lowering=False, debug=False,
                   num_devices=NCORES)

    # ---- DRAM parameters (per-core shard shapes) ----
    # d2 arrives host-pre-transposed per batch: [b, f-in-tile, kf, s]
    d2_d = nc.dram_tensor("d2", [BLOC, 128, 4, S], f16, kind="ExternalInput")
    d1t_d = nc.dram_tensor("d1t", [128, 4, BLOC], f16, kind="ExternalInput")
    w1_d = nc.dram_tensor("w1r", [128, 4, 2, 128], f16, kind="ExternalInput")
    wv_d = nc.dram_tensor("wvr", [128, 4, C], f16, kind="ExternalInput")
    wtop_d = nc.dram_tensor("wtopr", [128, H, 2, 2, 128], f16, kind="ExternalInput")
    wbot_d = nc.dram_tensor("wbotr", [128, H, 2, 2, 128], f16, kind="ExternalInput")
    pblk_d = nc.dram_tensor("pblkr", [128, 2 * H, H], f16, kind="ExternalInput")
    wcc_d = nc.dram_tensor("wccr", [128, 2 * H, OUTF], f16, kind="ExternalInput")
    bcct_d = nc.dram_tensor("bcct", [128, 1], f32, kind="ExternalInput")
    b1c_d = nc.dram_tensor("b1c", [128, 2], f32, kind="ExternalInput")
    id8_d = nc.dram_tensor("id8", [8, 8], f16, kind="ExternalInput")
    out_d = nc.dram_tensor("out", [OUTF, BLOC], f32, kind="ExternalOutput")

    with tile.TileContext(nc) as tc, ExitStack() as stk:
        const = stk.enter_context(tc.tile_pool(name="const", bufs=1))
        xtp = stk.enter_context(tc.tile_pool(name="xtp", bufs=3))
        d3p = stk.enter_context(tc.tile_pool(name="d3p", bufs=3))
        tvpool = stk.enter_context(tc.tile_pool(name="tvpool", bufs=2))
        usp = stk.enter_context(tc.tile_pool(name="usp", bufs=4))
        smallsb = stk.enter_context(tc.tile_pool(name="smallsb", bufs=2))
        vpool = stk.enter_context(tc.tile_pool(name="vpool", bufs=1))
        pmm = stk.enter_context(tc.tile_pool(name="pmm", bufs=2, space="PSUM"))
        pzs = stk.enter_context(tc.tile_pool(name="pzs", bufs=4, space="PSUM"))
        patp = stk.enter_context(tc.tile_pool(name="patp", bufs=1, space="PSUM"))
        psmall = stk.enter_context(
            tc.tile_pool(name="psmall", bufs=1, space="PSUM"))

        def load_xt(b, name):
            xt = xtp.tile([128, 4, S], f16, tag="xt", name=name)
            for half in range(2):
                nc.sync.dma_start(
                    out=xt[:, half * 2:(half + 1) * 2, :],
                    in_=d2_d[b, :, half * 2:(half + 1) * 2, :])
            return xt

        # All DMAs on the sync queue in near-consumption order.
        xt_tiles = {}
        w1_sb = const.tile([128, 4, 2, 128], f16, tag="w1")
        nc.sync.dma_start(out=w1_sb, in_=w1_d[:, :, :, :])
        xt_tiles[0] = load_xt(0, "xt_pre0")
        d1t_sb = const.tile([128, 4, BLOC], f16, tag="d1t")
        nc.sync.dma_start(out=d1t_sb, in_=d1t_d[:, :, :])
        b1c_sb = const.tile([128, 2], f32, tag="b1c")
        nc.sync.dma_start(out=b1c_sb, in_=b1c_d[:, :])
        wv_sb = const.tile([128, 4, C], f16, tag="wv")
        nc.sync.dma_start(out=wv_sb, in_=wv_d[:, :, :])
        xt_tiles[1] = load_xt(1, "xt_pre1")

        wbot_sb = const.tile([128, H, 2, 2, 128], f16, tag="wbot")
        for h in range(H):
            nc.sync.dma_start(out=wbot_sb[:, h, :, :, :],
                              in_=wbot_d[:, h, :, :, :])
        pblk_sb = const.tile([128, 2 * H, H], f16, tag="pblk")
        nc.sync.dma_start(out=pblk_sb, in_=pblk_d[:, :, :])
        wtop_sb = const.tile([128, H, 2, 2, 128], f16, tag="wtop")
        for h in range(H):
            nc.sync.dma_start(out=wtop_sb[:, h, :, :, :],
                              in_=wtop_d[:, h, :, :, :])
        id8_sb = const.tile([8, 8], f16, tag="id8")
        nc.sync.dma_start(out=id8_sb, in_=id8_d[:, :])
        bcct_sb = const.tile([128, 1], f32, tag="bcct")
        nc.sync.dma_start(out=bcct_sb, in_=bcct_d[:, :])
        wcc_sb = const.tile([128, 2 * H, OUTF], f16, tag="wcc")

        # ---- per-batch stage emitters ----
        d3ts = {}
        tvs = {}
        escs = {}
        zinvs = {}

        def emit_d3(b):
            xt = xt_tiles[b]
            d3t = d3p.tile([128, 2, S], f16, tag="d3t", name=f"d3t{b}")
            for m in range(2):
                pmd3 = pmm.tile([128, S], f32, tag="mm", name=f"pmd3_{b}_{m}")
                for kf in range(4):
                    nc.tensor.matmul(pmd3, lhsT=w1_sb[:, kf, m, :],
                                     rhs=xt[:, kf, :],
                                     start=(kf == 0), stop=(kf == 3))
                nc.scalar.activation(d3t[:, m, :], pmd3, AF.Relu,
                                     bias=b1c_sb[:, m:m + 1])
            d3ts[b] = d3t

        def emit_tv(b):
            xt = xt_tiles[b]
            tv = tvpool.tile([128, 4, C], f16, tag="tv", name=f"tv{b}")
            for mp in range(2):
                pmtv = pmm.tile([128, 2, C], f32, tag="mm",
                                name=f"pmtv_{b}_{mp}")
                for ms2 in range(2):
                    ms = mp * 2 + ms2
                    for kf in range(4):
                        nc.tensor.matmul(
                            pmtv[:, ms2, :],
                            lhsT=xt[:, kf, ms * 128:(ms + 1) * 128],
                            rhs=wv_sb[:, kf, :],
                            start=(kf == 0), stop=(kf == 3))
                nc.scalar.activation(tv[:, mp * 2:(mp + 1) * 2, :], pmtv,
                                     AF.Tanh)
            tvs[b] = tv

        def emit_heads(b):
            """zs/us/atts with atts one head behind zs (software pipeline),
            then the softmax front (nmax/exp/recip) on DVE+ACT."""
            d3t = d3ts[b]
            pat = patp.tile([8, S], f32, tag="atts", name=f"pat{b}")
            us_tiles = {}

            def emit_zs(h):
                us = usp.tile([128, 2, S], f16, tag="us", name=f"us{b}_{h}")
                for ct in range(2):
                    pz = pzs.tile([128, S], f32, tag="zs",
                                  name=f"pz{b}_{h}_{ct}")
                    for ks in range(2):
                        nc.tensor.matmul(pz, lhsT=wtop_sb[:, h, ks, ct, :],
                                         rhs=d3t[:, ks, :],
                                         start=(ks == 0), stop=(ks == 1))
                    nc.scalar.activation(us[:, ct, :], pz, AF.Tanh,
                                         bias=zc_sb[:, ct, h, b:b + 1])
                us_tiles[h] = us

            def emit_atts(h):
                us = us_tiles[h]
                for ct in range(2):
                    nc.tensor.matmul(pat, lhsT=pblk_sb[:, h * 2 + ct, :],
                                     rhs=us[:, ct, :],
                                     start=(h == 0 and ct == 0),
                                     stop=(h == H - 1 and ct == 1))

            emit_zs(0)
            for h in range(1, H):
                emit_zs(h)
                emit_atts(h - 1)
            emit_atts(H - 1)

            nmax = smallsb.tile([8, 1], f32, tag="nmax", name=f"nmax{b}")
            nc.vector.tensor_reduce(nmax, pat, axis=mybir.AxisListType.X,
                                    op=mybir.AluOpType.max, negate=True)
            esc = smallsb.tile([8, S], f16, tag="esc", name=f"esc{b}")
            zsum = smallsb.tile([8, 1], f32, tag="zsum", name=f"zsum{b}")
            nc.scalar.activation(esc, pat, AF.Exp, bias=nmax, accum_out=zsum)
            zinv = smallsb.tile([8, 1], f32, tag="zinv", name=f"zinv{b}")
            nc.vector.reciprocal(zinv, zsum)
            escs[b] = esc
            zinvs[b] = zinv

        def emit_tail(b):
            """scoresT transpose + vs + vsT into the V slab. Issued during
            batch b+1 so the softmax latency hides under d3(b+1)."""
            esc, zinv, tv = escs[b], zinvs[b], tvs[b]
            psc = psmall.tile([128, 4, 8], f16, tag="small", name=f"psc{b}")
            for sc in range(4):
                nc.tensor.transpose(psc[:, sc, :],
                                    in_=esc[:, sc * 128:(sc + 1) * 128],
                                    identity=id8_sb)
            sct = smallsb.tile([128, 4, 8], f16, tag="sct", name=f"sct{b}")
            nc.vector.tensor_copy(out=sct, in_=psc)
            pvs = psmall.tile([8, C], f32, tag="small", name=f"pvs{b}")
            for sc in range(4):
                nc.tensor.matmul(pvs, lhsT=sct[:, sc, :],
                                 rhs=tv[:, sc, :],
                                 start=(sc == 0), stop=(sc == 3))
            vssb = smallsb.tile([8, C], f16, tag="vssb", name=f"vssb{b}")
            nc.vector.tensor_scalar_mul(vssb, pvs, zinv)
            pvt = psmall.tile([128, 2, 8], f16, tag="small", name=f"pvt{b}")
            for ch in range(2):
                nc.tensor.transpose(pvt[:, ch, :],
                                    in_=vssb[:, ch * 128:(ch + 1) * 128],
                                    identity=id8_sb)
            for ch in range(2):
                nc.vector.tensor_copy(out=v_sb[:, ch, :, b:b + 1],
                                      in_=pvt[:, ch, :])

        # ---- V accumulator across the b loop ----
        v_sb = vpool.tile([128, 2, H, BLOC], f16)  # [c-in-half, ch, h, b]

        # ---- batch 0: d3 first (needs only xt0+w1, the earliest DMAs),
        # then the d4/zconst prologue (needs d1t/wbot, landing later).
        emit_d3(0)

        # d4T = relu(w1.T @ d1T + b1) : [C(2 tiles), BLOC]
        pd4 = psmall.tile([128, 2, BLOC], f32, tag="small")
        for m in range(2):
            for k in range(4):
                nc.tensor.matmul(pd4[:, m, :], lhsT=w1_sb[:, k, m, :],
                                 rhs=d1t_sb[:, k, :],
                                 start=(k == 0), stop=(k == 3))
        d4t_sb = const.tile([128, 2, BLOC], f16, tag="d4t")
        for m in range(2):
            nc.scalar.activation(d4t_sb[:, m, :], pd4[:, m, :], AF.Relu,
                                 bias=b1c_sb[:, m:m + 1])

        emit_tv(0)

        pzc = psmall.tile([128, 2, H, BLOC], f32, tag="small")
        for ct in range(2):
            for h in range(H):
                for ks in range(2):
                    nc.tensor.matmul(pzc[:, ct, h, :],
                                     lhsT=wbot_sb[:, h, ks, ct, :],
                                     rhs=d4t_sb[:, ks, :],
                                     start=(ks == 0), stop=(ks == 1))
        zc_sb = const.tile([128, 2, H, BLOC], f32, tag="zc")
        nc.vector.tensor_copy(out=zc_sb, in_=pzc)

        emit_heads(0)

        outsb = smallsb.tile([OUTF, BLOC], f32, tag="outsb")

        def emit_final(bs, be, name):
            """outT[:, bs:be] = relu(wcc.T @ V[:, bs:be] + bccT). Column b
            of the output depends only on batch b, so batches 0..6 project
            during batch 7's compute and only b7's column remains at the
            end."""
            pout = psmall.tile([OUTF, be - bs], f32, tag="small", name=name)
            kidx = 0
            for h in range(H):
                for ch in range(2):
                    nc.tensor.matmul(pout, lhsT=wcc_sb[:, h * 2 + ch, :],
                                     rhs=v_sb[:, ch, h, bs:be],
                                     start=(kidx == 0), stop=(kidx == 15))
                    kidx += 1
            nc.scalar.activation(outsb[:, bs:be], pout, AF.Relu,
                                 bias=bcct_sb[:, 0:1])

        for b in range(1, BLOC):
            if b + 1 < BLOC:
                xt_tiles[b + 1] = load_xt(b + 1, f"xt{b + 1}")
            if b == 2:
                nc.sync.dma_start(out=wcc_sb, in_=wcc_d[:, :, :])
            emit_d3(b)
            emit_tail(b - 1)
            if b < BLOC - 1:
                emit_tv(b)
                emit_heads(b)
        # last batch: heads first, then tv + the b0..6 projection fill the
        # softmax latency; only b7's rank-1 column remains after tail(7).
        emit_heads(BLOC - 1)
        emit_tv(BLOC - 1)
        emit_final(0, BLOC - 1, "pout_a")
        emit_tail(BLOC - 1)
        emit_final(BLOC - 1, BLOC, "pout_b")
        nc.sync.dma_start(out=out_d[:, :], in_=outsb)

    nc.compile()
    return nc


def host_inputs(d1, d2, w1, b1, W, P, wv, wcc, bcc):
    """Host-side sharding + layout prep. Returns in_maps for 8 cores."""
    d1 = np.ascontiguousarray(d1, dtype=np.float32)
    d2 = np.ascontiguousarray(d2, dtype=np.float32)
    w1 = np.ascontiguousarray(w1, dtype=np.float32)
    b1 = np.ascontiguousarray(b1, dtype=np.float32)
    W = np.ascontiguousarray(W, dtype=np.float32)
    P = np.ascontiguousarray(P, dtype=np.float32)
    wv = np.ascontiguousarray(wv, dtype=np.float32)
    wcc = np.ascontiguousarray(wcc, dtype=np.float32)
    bcc = np.ascontiguousarray(bcc, dtype=np.float32)

    w1r = np.ascontiguousarray(
        w1.reshape(4, 128, 2, 128).transpose(1, 0, 2, 3))
    wvr = np.ascontiguousarray(wv.reshape(4, 128, C).transpose(1, 0, 2))
    wtopr = np.ascontiguousarray(
        W[:, :C, :].reshape(H, 2, 128, 2, 128).transpose(2, 0, 1, 3, 4))
    wbotr = np.ascontiguousarray(
        W[:, C:, :].reshape(H, 2, 128, 2, 128).transpose(2, 0, 1, 3, 4))
    pblkr = np.zeros((128, 2 * H, H), np.float32)
    for h in range(H):
        for ct in range(2):
            pblkr[:, h * 2 + ct, h] = P[h, ct * 128:(ct + 1) * 128]
    wccr = np.ascontiguousarray(
        wcc.reshape(2 * H, 128, OUTF).transpose(1, 0, 2))
    bcct = np.ascontiguousarray(bcc[:, None])
    b1c = np.ascontiguousarray(b1.reshape(2, 128).T)
    id8 = np.eye(8, dtype=np.float32)

    f16 = np.float16
    shared = dict(w1r=w1r.astype(f16), wvr=wvr.astype(f16),
                  wtopr=wtopr.astype(f16), wbotr=wbotr.astype(f16),
                  pblkr=pblkr.astype(f16), wccr=wccr.astype(f16),
                  bcct=bcct, b1c=b1c, id8=id8.astype(f16))
    # pre-transpose d2 on host: [S, B, F] -> per-core [b, f-in-tile, kf, s]
    d2t = np.ascontiguousarray(
        d2.transpose(1, 2, 0).astype(np.float16))  # [B, F, S]
    in_maps = []
    for core in range(NCORES):
        bs = slice(core * BLOC, (core + 1) * BLOC)
        d2c = np.ascontiguousarray(
            d2t[bs].reshape(BLOC, 4, 128, S).transpose(0, 2, 1, 3))
        d1c = d1[bs]  # [BLOC, F]
        d1tr = np.ascontiguousarray(
            d1c.T.reshape(4, 128, BLOC).transpose(1, 0, 2)).astype(np.float16)
        in_maps.append(dict(d2=d2c, d1t=d1tr, **shared))
    return in_maps


def kernel(**inputs):
    if "nc" not in _CACHE:
        _CACHE["nc"] = build_nc()
    nc = _CACHE["nc"]
    in_maps = host_inputs(
        d1=inputs["d1"], d2=inputs["d2"], w1=inputs["w1"], b1=inputs["b1"],
        W=inputs["W"], P=inputs["P"], wv=inputs["wv"], wcc=inputs["wcc"],
        bcc=inputs["bcc"])
    from concourse.bass_utils import run_bass_kernel_spmd
    res = run_bass_kernel_spmd(nc, in_maps, core_ids=list(range(NCORES)))
    return np.concatenate(
        [res.results[i]["out"].T for i in range(NCORES)], axis=0)
